# revision 12
# baseline (speedup 1.0000x reference)
"""Radix-4 DIF ambiguity kernel, bf16 lag products, host-folded normalization.

X[k, 4t+j] = sum_{m<256} B_j[m, k] * (w1024^{jm} w256^{mt})   (tables, bf16)
B_0 =  (R0+R2) + (R1+R3)     B_2 = (R0+R2) - (R1+R3)
B_1 =  (R0-R2) - i(R1-R3)    B_3 = (R0-R2) + i(R1-R3)     (Rl = R[m+256l])

The ambiguity max is always at the origin (Cauchy-Schwarz), so the max-
normalization is folded into a host-side input scaling s -> s/sqrt(sum|s|^2).
Lag products via 3-op STT chains against host-precomputed difference/sum
buffers. Only k in [0,512] is computed; rows 1..511 come from the
chi(-k,-f) symmetry: f-reversal on DVE, k-reversal via J-flip matmul.
"""

import numpy as np
import ml_dtypes

import bass_rust
import concourse.bass as bass
import concourse.mybir as mybir
import concourse.tile as tile
import concourse.bass_utils as bass_utils

B, N = 16, 1024
NCORES = 8
BPC = B // NCORES
KW = 520  # k-width computed: kb0-3 full, kb4 holds k=512 (+7 pad)

f32 = mybir.dt.float32
f32r = mybir.dt.float32r
bf16 = mybir.dt.bfloat16
ALU = mybir.AluOpType

bf16np = ml_dtypes.bfloat16


def _split_excess_waits(nc):
    for f in nc.m.functions:
        for blk in f.blocks:
            insts = list(blk.instructions)
            new_insts = []
            changed = False
            for inst in insts:
                si = inst.sync_info
                waits = list(si.on_wait) if (si is not None and si.on_wait) else []
                keep_n = 0 if isinstance(inst, mybir.InstDrain) else 1
                if len(waits) > keep_n:
                    changed = True
                    extra = waits[: len(waits) - keep_n]
                    keep = waits[len(waits) - keep_n:]
                    for w in extra:
                        nop = mybir.InstNoOp(
                            name=nc.get_next_instruction_name(), ins=[], outs=[]
                        )
                        nop.engine = inst.engine
                        nop.sync_info = bass_rust.SyncInfo(on_wait=[w], on_update=[])
                        new_insts.append(nop)
                    inst.sync_info = bass_rust.SyncInfo(
                        on_wait=keep,
                        on_update=list(si.on_update) if si.on_update else [],
                    )
                new_insts.append(inst)
            if changed:
                blk.instructions = new_insts
    return nc


def build_nc():
    nc = bass.Bass("TRN2", target_bir_lowering=False, debug=False)

    dsr = nc.dram_tensor("dsr", [BPC, 2048], bf16, kind="ExternalInput")
    dD = nc.dram_tensor("dD", [BPC, 2048], bf16, kind="ExternalInput")
    dSn = nc.dram_tensor("dSn", [BPC, 2048], bf16, kind="ExternalInput")
    scols = nc.dram_tensor("scols", [BPC, 128, 24], f32, kind="ExternalInput")
    tabs = {}
    for j in range(4):
        for kind in ("c", "s", "sn"):
            nm = f"t{j}{kind}"
            tabs[(j, kind)] = nc.dram_tensor(nm, [256, 256], bf16, kind="ExternalInput")
    jmat = nc.dram_tensor("jmat", [128, 128], f32r, kind="ExternalInput")
    out = nc.dram_tensor("out", [BPC, N, N], f32, kind="ExternalOutput")

    with tile.TileContext(nc) as tc:
        with (
            tc.tile_pool(name="const", bufs=1) as constp,
            tc.tile_pool(name="win", bufs=2) as winp,
            tc.tile_pool(name="sm", bufs=2) as smp,
            tc.tile_pool(name="u", bufs=1) as up,
            tc.tile_pool(name="tmp", bufs=2) as tmpp,
            tc.tile_pool(name="r1", bufs=1) as r1p,
            tc.tile_pool(name="b2", bufs=2) as b2p,
            tc.tile_pool(name="sq", bufs=2) as sqp,
            tc.tile_pool(name="chi", bufs=4) as chip,
            tc.tile_pool(name="rev", bufs=1) as revp,
            tc.tile_pool(name="mj", bufs=1) as mjp,
            tc.tile_pool(name="ps", bufs=2, space="PSUM") as psp,
        ):
            tJ = constp.tile([128, 128], f32r, tag="jmat")
            nc.scalar.dma_start(tJ[:], jmat[:])
            TT = {}
            for j in range(4):
                for kind in ("c", "s", "sn"):
                    for c in range(2):
                        TT[(j, kind, c)] = constp.tile(
                            [128, 256], bf16, tag=f"t{j}{kind}{c}",
                            name=f"tt{j}{kind}{c}",
                        )

            def load_tab(j, kind, c, eng):
                eng.dma_start(
                    TT[(j, kind, c)][:], tabs[(j, kind)][128 * c:128 * (c + 1), :]
                )

            def emit_load(b):
                s = {"b": b, "chi": {}, "u": {}, "Rs": {}, "Rd": {}, "B": {}}
                scol = smp.tile([128, 24], f32, tag="scol")
                nc.sync.dma_start(scol[:], scols[b])
                Tsr = winp.tile([128, 1921], bf16, tag="tsr")
                TD = winp.tile([128, 1921], bf16, tag="tD")
                TSn = winp.tile([128, 1921], bf16, tag="tSn")
                # row p = s_tiled[p : p+1921] (shift-by-1 rows via DRAM stride)
                nc.sync.dma_start(Tsr[:], bass.AP(dsr, b * 2048, [[1, 128], [1, 1921]]))
                nc.sync.dma_start(TD[:], bass.AP(dD, b * 2048, [[1, 128], [1, 1921]]))
                nc.sync.dma_start(TSn[:], bass.AP(dSn, b * 2048, [[1, 128], [1, 1921]]))
                s["scol"] = scol
                s["T"] = (Tsr, TD, TSn)
                return s

            def win(T, q8, lo, n):
                # [p, kk] -> s_tiled[1024 + 128*q8 + p - (lo+kk)], kk in [0, n)
                ap = T[:]
                return bass.AP(
                    ap.tensor, ap.offset + 1024 + 128 * q8 - lo, [ap.ap[0], [-1, n]]
                )

            def emit_product(s, q8, lo, hi):
                # u = s[m] * conj(s)[(m-k)%N]:
                #   t    = w_sr * (sr+si)_m          (ACT)
                #   u_re = w_(si-sr) * si_m + t      (DVE/Pool STT)
                #   u_im = w_-(sr+si) * sr_m + t
                Tsr, TD, TSn = s["T"]
                scol = s["scol"]
                n = hi - lo
                sr_c = scol[:, q8:q8 + 1]
                si_c = scol[:, 8 + q8:9 + q8]
                ss_c = scol[:, 16 + q8:17 + q8]
                t = tmpp.tile([128, KW], bf16, tag=f"t{q8 % 2}")
                nc.scalar.mul(t[:, lo:hi], win(Tsr, q8, lo, n), ss_c)
                if lo == 0:
                    ur = up.tile([128, KW], bf16, tag=f"ur{q8}")
                    ui = up.tile([128, KW], bf16, tag=f"ui{q8}")
                    s["u"][q8] = (ur, ui)
                ur, ui = s["u"][q8]
                nc.vector.scalar_tensor_tensor(
                    ur[:, lo:hi], win(TD, q8, lo, n), si_c, t[:, lo:hi],
                    op0=ALU.mult, op1=ALU.add,
                )
                nc.vector.scalar_tensor_tensor(
                    ui[:, lo:hi], win(TSn, q8, lo, n), sr_c, t[:, lo:hi],
                    op0=ALU.mult, op1=ALU.add,
                )

            def emit_L1(s, c, lo, hi):
                # Rsum/Rdiff over m vs m+512: pairs (c, c+4), c in 0..3  (Pool)
                u1r, u1i = s["u"][c]
                u2r, u2i = s["u"][c + 4]
                if lo == 0:
                    rsr = r1p.tile([128, KW], bf16, tag=f"rsr{c}")
                    rsi = r1p.tile([128, KW], bf16, tag=f"rsi{c}")
                    rdr = r1p.tile([128, KW], bf16, tag=f"rdr{c}")
                    rdi = r1p.tile([128, KW], bf16, tag=f"rdi{c}")
                    s["Rs"][c] = (rsr, rsi)
                    s["Rd"][c] = (rdr, rdi)
                (rsr, rsi), (rdr, rdi) = s["Rs"][c], s["Rd"][c]
                sl = slice(lo, hi)
                eng = nc.gpsimd if c < 2 else nc.vector
                eng.tensor_add(rsr[:, sl], u1r[:, sl], u2r[:, sl])
                eng.tensor_add(rsi[:, sl], u1i[:, sl], u2i[:, sl])
                eng.tensor_sub(rdr[:, sl], u1r[:, sl], u2r[:, sl])
                eng.tensor_sub(rdi[:, sl], u1i[:, sl], u2i[:, sl])

            def emit_L2(s, c, lo, hi):
                # B_j over m'' vs m''+256: pairs (c, c+2), c in 0..1  (DVE)
                asr, asi = s["Rs"][c]
                bsr, bsi = s["Rs"][c + 2]
                cdr, cdi = s["Rd"][c]
                ddr, ddi = s["Rd"][c + 2]
                sl = slice(lo, hi)
                for j, (x, y, op0, op1) in {
                    0: ((asr, asi), (bsr, bsi), ALU.add, ALU.add),
                    2: ((asr, asi), (bsr, bsi), ALU.subtract, ALU.subtract),
                    # B1 = c - i d: re = cr + di ; im = ci - dr
                    1: ((cdr, cdi), (ddi, ddr), ALU.add, ALU.subtract),
                    # B3 = c + i d: re = cr - di ; im = ci + dr
                    3: ((cdr, cdi), (ddi, ddr), ALU.subtract, ALU.add),
                }.items():
                    if lo == 0:
                        bre = b2p.tile([128, KW], bf16, tag=f"b{j}re{c}")
                        bim = b2p.tile([128, KW], bf16, tag=f"b{j}im{c}")
                        s["B"][(j, c)] = (bre, bim)
                    bre, bim = s["B"][(j, c)]
                    nc.vector.tensor_tensor(bre[:, sl], x[0][:, sl], y[0][:, sl], op=op0)
                    nc.vector.tensor_tensor(bim[:, sl], x[1][:, sl], y[1][:, sl], op=op1)

            def emit_kblock(s, kb):
                c0 = 128 * kb
                kwid = 128 if kb < 4 else 8
                pslc = slice(0, kwid)
                xt = {}
                for h in range(2):
                    xt[("re", h)] = psp.tile(
                        [128, 512], f32, tag=f"xre{h}", name=f"xre{h}_{kb}"
                    )
                    xt[("im", h)] = psp.tile(
                        [128, 512], f32, tag=f"xim{h}", name=f"xim{h}_{kb}"
                    )
                for j in range(4):
                    h, o = j // 2, 256 * (j % 2)
                    xre = xt[("re", h)][pslc, o:o + 256]
                    xim = xt[("im", h)][pslc, o:o + 256]
                    for c in range(2):
                        bre, bim = s["B"][(j, c)]
                        first, last = c == 0, c == 1
                        psr = bre[:, c0:c0 + kwid]
                        psi = bim[:, c0:c0 + kwid]
                        nc.tensor.matmul(xre, psr, TT[(j, "c", c)][:], start=first, stop=False)
                        nc.tensor.matmul(xim, psi, TT[(j, "c", c)][:], start=first, stop=False)
                        nc.tensor.matmul(xre, psi, TT[(j, "s", c)][:], start=False, stop=last)
                        nc.tensor.matmul(xim, psr, TT[(j, "sn", c)][:], start=False, stop=last)
                chi_t = chip.tile([128, N], f32, tag="chi")
                for h in range(2):
                    sqr = sqp.tile([128, 512], f32, tag=f"sqr{h}")
                    sqi = sqp.tile([128, 512], f32, tag=f"sqi{h}")
                    nc.scalar.square(sqr[pslc, :], xt[("re", h)][pslc, :])
                    nc.scalar.square(sqi[pslc, :], xt[("im", h)][pslc, :])
                    for jh in range(2):
                        j = 2 * h + jh
                        o = 256 * jh
                        cap = chi_t[pslc, :]
                        strided = bass.AP(
                            cap.tensor, cap.offset + j, [cap.ap[0], [4, 256]]
                        )
                        nc.gpsimd.tensor_add(
                            strided, sqr[pslc, o:o + 256], sqi[pslc, o:o + 256]
                        )
                s["chi"][kb] = chi_t

            def emit_store_direct(s, kb):
                b = s["b"]
                chi_t = s["chi"][kb]
                if kb < 4:
                    nc.sync.dma_start(out[b, 512 + 128 * kb:640 + 128 * kb, :], chi_t[:])
                else:
                    nc.sync.dma_start(out[b, 0:1, :], chi_t[0:1, :])

            def emit_mirror(s, kb):
                # chi_rev = [chi[0], chi[1023..1]] then J-flip rows via PE
                chi_t = s["chi"][kb]
                cr = revp.tile([128, N], f32r, tag=f"rev{kb % 2}")
                cap = chi_t[:]
                nc.vector.tensor_copy(cr[:, 0:1], chi_t[:, 0:1])
                rsrc = bass.AP(cap.tensor, cap.offset + 1023, [cap.ap[0], [-1, 1023]])
                nc.vector.tensor_copy(cr[:, 1:1024], rsrc)
                mj = mjp.tile([128, N], f32, tag=f"mj{kb % 2}")
                for h in range(2):
                    jy = psp.tile([128, 512], f32, tag=f"xre{h}", name=f"jy{h}_{kb}")
                    nc.tensor.matmul(
                        jy[:], tJ[:], cr[:, 512 * h:512 * h + 512], start=True, stop=True
                    )
                    nc.scalar.copy(mj[:, 512 * h:512 * h + 512], jy[:])
                s["mj"] = s.get("mj", {})
                s["mj"][kb] = mj

            def emit_store_mirror(s, kb):
                # mj partition p holds k1 = 128*kb + 127 - p -> out row 385-128*kb+p
                b = s["b"]
                mj = s["mj"][kb]
                if kb == 0:
                    nc.sync.dma_start(out[b, 385:512, :], mj[0:127, :])
                else:
                    r0 = 385 - 128 * kb
                    nc.sync.dma_start(out[b, r0:r0 + 128, :], mj[:])

            # ---- schedule: sliced rbuild for batch 0 so PE starts early
            s0 = emit_load(0)
            for j in range(4):
                for kind in ("c", "s", "sn"):
                    load_tab(j, kind, 0, nc.scalar)
                    load_tab(j, kind, 1, nc.scalar)
            SA, SB = (0, 256), (256, KW)
            for c in range(4):
                emit_product(s0, c, *SA)
                emit_product(s0, c + 4, *SA)
                emit_L1(s0, c, *SA)
            emit_L2(s0, 0, *SA)
            emit_L2(s0, 1, *SA)
            emit_kblock(s0, 0)
            for c in range(4):
                emit_product(s0, c, *SB)
                emit_product(s0, c + 4, *SB)
                emit_L1(s0, c, *SB)
            emit_kblock(s0, 1)
            emit_mirror(s0, 0)
            emit_store_direct(s0, 0)
            emit_store_mirror(s0, 0)
            emit_L2(s0, 0, *SB)
            emit_L2(s0, 1, *SB)
            s1 = emit_load(1)
            emit_kblock(s0, 2)
            emit_mirror(s0, 1)
            emit_store_direct(s0, 1)
            emit_store_mirror(s0, 1)
            for c in (0, 4, 1, 5):
                emit_product(s1, c, 0, KW)
            emit_kblock(s0, 3)
            emit_mirror(s0, 2)
            emit_store_direct(s0, 2)
            emit_store_mirror(s0, 2)
            emit_L1(s1, 0, 0, KW)
            emit_L1(s1, 1, 0, KW)
            for c in (2, 6, 3, 7):
                emit_product(s1, c, 0, KW)
            emit_kblock(s0, 4)
            emit_mirror(s0, 3)
            emit_store_direct(s0, 3)
            emit_store_mirror(s0, 3)
            emit_store_direct(s0, 4)
            emit_L1(s1, 2, 0, KW)
            emit_L1(s1, 3, 0, KW)
            emit_L2(s1, 0, 0, KW)
            emit_L2(s1, 1, 0, KW)
            emit_kblock(s1, 0)
            emit_mirror(s1, 0)
            emit_store_direct(s1, 0)
            emit_store_mirror(s1, 0)
            emit_kblock(s1, 1)
            emit_mirror(s1, 1)
            emit_store_direct(s1, 1)
            emit_store_mirror(s1, 1)
            emit_kblock(s1, 2)
            emit_mirror(s1, 2)
            emit_store_direct(s1, 2)
            emit_store_mirror(s1, 2)
            emit_kblock(s1, 3)
            emit_mirror(s1, 3)
            emit_store_direct(s1, 3)
            emit_store_mirror(s1, 3)
            emit_kblock(s1, 4)
            emit_store_direct(s1, 4)

    _split_excess_waits(nc)
    return nc


_NC_CACHE = {}


def _get_nc():
    if "nc" not in _NC_CACHE:
        _NC_CACHE["nc"] = build_nc()
    return _NC_CACHE["nc"]


def _get_tables():
    if "tabs" not in _NC_CACHE:
        m = np.arange(256, dtype=np.float64)[:, None]
        tc_ = np.arange(256, dtype=np.float64)[None, :]
        t_of = (tc_ + 128) % 256
        tabs = {}
        for j in range(4):
            ang = 2.0 * np.pi * (m * j / 1024.0 + (m * t_of) % 256 / 256.0)
            tabs[f"t{j}c"] = np.cos(ang).astype(bf16np)
            tabs[f"t{j}s"] = np.sin(ang).astype(bf16np)
            tabs[f"t{j}sn"] = (-np.sin(ang)).astype(bf16np)
        _NC_CACHE["tabs"] = (tabs, np.eye(128, dtype=np.float32)[::-1].copy())
    return _NC_CACHE["tabs"]


def _make_in_maps(s_real, s_imag):
    s_real = np.asarray(s_real, dtype=np.float32)
    s_imag = np.asarray(s_imag, dtype=np.float32)
    tabs, jnp_ = _get_tables()
    E = (
        s_real.astype(np.float64) ** 2 + s_imag.astype(np.float64) ** 2
    ).sum(axis=1, keepdims=True)
    scale = E ** -0.5
    srn = (s_real * scale).astype(np.float32)
    sin_ = (s_imag * scale).astype(np.float32)

    in_maps = []
    for core in range(NCORES):
        sl = slice(core * BPC, (core + 1) * BPC)
        sr = srn[sl]
        si = sin_[sl]
        dsr = np.tile(sr, (1, 2)).astype(bf16np)
        dD = np.tile(si - sr, (1, 2)).astype(bf16np)
        dSn = np.tile(-(sr + si), (1, 2)).astype(bf16np)
        scols = np.concatenate(
            [
                sr.reshape(BPC, 8, 128).transpose(0, 2, 1),
                si.reshape(BPC, 8, 128).transpose(0, 2, 1),
                (sr + si).reshape(BPC, 8, 128).transpose(0, 2, 1),
            ],
            axis=2,
        ).astype(np.float32).copy()
        im = {"dsr": dsr, "dD": dD, "dSn": dSn, "scols": scols, "jmat": jnp_}
        im.update(tabs)
        in_maps.append(im)
    return in_maps


def kernel(s_real: np.ndarray, s_imag: np.ndarray) -> np.ndarray:
    nc = _get_nc()
    in_maps = _make_in_maps(s_real, s_imag)
    res = bass_utils.run_bass_kernel_spmd(nc, in_maps, core_ids=list(range(NCORES)))
    return np.concatenate([r["out"] for r in res.results], axis=0)


# revision 14
# speedup vs baseline: 1.1226x; 1.1226x over previous
"""Radix-4 DIF ambiguity kernel, bf16 lag products, host-folded normalization.

X[k, 4t+j] = sum_{m<256} B_j[m, k] * (w1024^{jm} w256^{mt})   (tables, bf16)
B_0 =  (R0+R2) + (R1+R3)     B_2 = (R0+R2) - (R1+R3)
B_1 =  (R0-R2) - i(R1-R3)    B_3 = (R0-R2) + i(R1-R3)     (Rl = R[m+256l])

The ambiguity max is always at the origin (Cauchy-Schwarz), so the max-
normalization is folded into a host-side input scaling s -> s/sqrt(sum|s|^2).
Lag products via 3-op STT chains against host-precomputed difference/sum
buffers. Only k in [0,512] is computed; rows 1..511 come from the
chi(-k,-f) symmetry: f-reversal on DVE, k-reversal via J-flip matmul.
"""

import numpy as np
import ml_dtypes

import bass_rust
import concourse.bass as bass
import concourse.mybir as mybir
import concourse.tile as tile
import concourse.bass_utils as bass_utils

B, N = 16, 1024
NCORES = 8
BPC = B // NCORES
KW = 520  # k-width computed: kb0-3 full, kb4 holds k=512 (+7 pad)

f32 = mybir.dt.float32
f32r = mybir.dt.float32r
bf16 = mybir.dt.bfloat16
ALU = mybir.AluOpType

bf16np = ml_dtypes.bfloat16


def _split_excess_waits(nc):
    for f in nc.m.functions:
        for blk in f.blocks:
            insts = list(blk.instructions)
            new_insts = []
            changed = False
            for inst in insts:
                si = inst.sync_info
                waits = list(si.on_wait) if (si is not None and si.on_wait) else []
                keep_n = 0 if isinstance(inst, mybir.InstDrain) else 1
                if len(waits) > keep_n:
                    changed = True
                    extra = waits[: len(waits) - keep_n]
                    keep = waits[len(waits) - keep_n:]
                    for w in extra:
                        nop = mybir.InstNoOp(
                            name=nc.get_next_instruction_name(), ins=[], outs=[]
                        )
                        nop.engine = inst.engine
                        nop.sync_info = bass_rust.SyncInfo(on_wait=[w], on_update=[])
                        new_insts.append(nop)
                    inst.sync_info = bass_rust.SyncInfo(
                        on_wait=keep,
                        on_update=list(si.on_update) if si.on_update else [],
                    )
                new_insts.append(inst)
            if changed:
                blk.instructions = new_insts
    return nc


def build_nc():
    nc = bass.Bass("TRN2", target_bir_lowering=False, debug=False)

    dsr = nc.dram_tensor("dsr", [BPC, 2048], bf16, kind="ExternalInput")
    dD = nc.dram_tensor("dD", [BPC, 2048], bf16, kind="ExternalInput")
    dSn = nc.dram_tensor("dSn", [BPC, 2048], bf16, kind="ExternalInput")
    scols = nc.dram_tensor("scols", [BPC, 128, 24], f32, kind="ExternalInput")
    tabs = {}
    for j in range(4):
        for kind in ("c", "s", "sn"):
            nm = f"t{j}{kind}"
            tabs[(j, kind)] = nc.dram_tensor(nm, [256, 256], bf16, kind="ExternalInput")
    jmat = nc.dram_tensor("jmat", [128, 128], f32r, kind="ExternalInput")
    out = nc.dram_tensor("out", [BPC, N, N], f32, kind="ExternalOutput")

    with tile.TileContext(nc) as tc:
        with (
            tc.tile_pool(name="const", bufs=1) as constp,
            tc.tile_pool(name="win", bufs=2) as winp,
            tc.tile_pool(name="sm", bufs=2) as smp,
            tc.tile_pool(name="u", bufs=1) as up,
            tc.tile_pool(name="tmp", bufs=2) as tmpp,
            tc.tile_pool(name="r1", bufs=1) as r1p,
            tc.tile_pool(name="b2", bufs=2) as b2p,
            tc.tile_pool(name="sq", bufs=2) as sqp,
            tc.tile_pool(name="chi", bufs=4) as chip,
            tc.tile_pool(name="rev", bufs=1) as revp,
            tc.tile_pool(name="mj", bufs=1) as mjp,
            tc.tile_pool(name="ps", bufs=2, space="PSUM") as psp,
        ):
            tJ = constp.tile([128, 128], f32r, tag="jmat")
            nc.sync.dma_start(tJ[:], jmat[:])
            TT = {}
            for j in range(4):
                for kind in ("c", "s", "sn"):
                    for c in range(2):
                        TT[(j, kind, c)] = constp.tile(
                            [128, 256], bf16, tag=f"t{j}{kind}{c}",
                            name=f"tt{j}{kind}{c}",
                        )

            def load_tab(j, kind, c, eng):
                eng.dma_start(
                    TT[(j, kind, c)][:], tabs[(j, kind)][128 * c:128 * (c + 1), :]
                )

            def emit_load(b):
                s = {"b": b, "chi": {}, "u": {}, "Rs": {}, "Rd": {}, "B": {}}
                scol = smp.tile([128, 24], f32, tag="scol")
                nc.sync.dma_start(scol[:], scols[b])
                Tsr = winp.tile([128, 1921], bf16, tag="tsr")
                TD = winp.tile([128, 1921], bf16, tag="tD")
                TSn = winp.tile([128, 1921], bf16, tag="tSn")
                # row p = s_tiled[p : p+1921] (shift-by-1 rows via DRAM stride)
                nc.sync.dma_start(Tsr[:], bass.AP(dsr, b * 2048, [[1, 128], [1, 1921]]))
                nc.sync.dma_start(TD[:], bass.AP(dD, b * 2048, [[1, 128], [1, 1921]]))
                nc.sync.dma_start(TSn[:], bass.AP(dSn, b * 2048, [[1, 128], [1, 1921]]))
                s["scol"] = scol
                s["T"] = (Tsr, TD, TSn)
                return s

            def win(T, q8, lo, n):
                # [p, kk] -> s_tiled[1024 + 128*q8 + p - (lo+kk)], kk in [0, n)
                ap = T[:]
                return bass.AP(
                    ap.tensor, ap.offset + 1024 + 128 * q8 - lo, [ap.ap[0], [-1, n]]
                )

            def emit_product(s, q8, lo, hi):
                # u = s[m] * conj(s)[(m-k)%N]:
                #   t    = w_sr * (sr+si)_m          (ACT)
                #   u_re = w_(si-sr) * si_m + t      (DVE/Pool STT)
                #   u_im = w_-(sr+si) * sr_m + t
                Tsr, TD, TSn = s["T"]
                scol = s["scol"]
                n = hi - lo
                sr_c = scol[:, q8:q8 + 1]
                si_c = scol[:, 8 + q8:9 + q8]
                ss_c = scol[:, 16 + q8:17 + q8]
                t = tmpp.tile([128, KW], bf16, tag=f"t{q8 % 2}")
                nc.scalar.mul(t[:, lo:hi], win(Tsr, q8, lo, n), ss_c)
                if lo == 0:
                    ur = up.tile([128, KW], bf16, tag=f"ur{q8}")
                    ui = up.tile([128, KW], bf16, tag=f"ui{q8}")
                    s["u"][q8] = (ur, ui)
                ur, ui = s["u"][q8]
                nc.vector.scalar_tensor_tensor(
                    ur[:, lo:hi], win(TD, q8, lo, n), si_c, t[:, lo:hi],
                    op0=ALU.mult, op1=ALU.add,
                )
                nc.vector.scalar_tensor_tensor(
                    ui[:, lo:hi], win(TSn, q8, lo, n), sr_c, t[:, lo:hi],
                    op0=ALU.mult, op1=ALU.add,
                )

            def emit_L1(s, c, lo, hi):
                # Rsum/Rdiff over m vs m+512: pairs (c, c+4), c in 0..3  (Pool)
                u1r, u1i = s["u"][c]
                u2r, u2i = s["u"][c + 4]
                if lo == 0:
                    rsr = r1p.tile([128, KW], bf16, tag=f"rsr{c}")
                    rsi = r1p.tile([128, KW], bf16, tag=f"rsi{c}")
                    rdr = r1p.tile([128, KW], bf16, tag=f"rdr{c}")
                    rdi = r1p.tile([128, KW], bf16, tag=f"rdi{c}")
                    s["Rs"][c] = (rsr, rsi)
                    s["Rd"][c] = (rdr, rdi)
                (rsr, rsi), (rdr, rdi) = s["Rs"][c], s["Rd"][c]
                sl = slice(lo, hi)
                eng = nc.gpsimd if c < 2 else nc.vector
                eng.tensor_add(rsr[:, sl], u1r[:, sl], u2r[:, sl])
                eng.tensor_add(rsi[:, sl], u1i[:, sl], u2i[:, sl])
                eng.tensor_sub(rdr[:, sl], u1r[:, sl], u2r[:, sl])
                eng.tensor_sub(rdi[:, sl], u1i[:, sl], u2i[:, sl])

            def emit_L2(s, c, lo, hi):
                # B_j over m'' vs m''+256: pairs (c, c+2), c in 0..1  (DVE)
                asr, asi = s["Rs"][c]
                bsr, bsi = s["Rs"][c + 2]
                cdr, cdi = s["Rd"][c]
                ddr, ddi = s["Rd"][c + 2]
                sl = slice(lo, hi)
                for j, (x, y, op0, op1) in {
                    0: ((asr, asi), (bsr, bsi), ALU.add, ALU.add),
                    2: ((asr, asi), (bsr, bsi), ALU.subtract, ALU.subtract),
                    # B1 = c - i d: re = cr + di ; im = ci - dr
                    1: ((cdr, cdi), (ddi, ddr), ALU.add, ALU.subtract),
                    # B3 = c + i d: re = cr - di ; im = ci + dr
                    3: ((cdr, cdi), (ddi, ddr), ALU.subtract, ALU.add),
                }.items():
                    if lo == 0:
                        bre = b2p.tile([128, KW], bf16, tag=f"b{j}re{c}")
                        bim = b2p.tile([128, KW], bf16, tag=f"b{j}im{c}")
                        s["B"][(j, c)] = (bre, bim)
                    bre, bim = s["B"][(j, c)]
                    nc.vector.tensor_tensor(bre[:, sl], x[0][:, sl], y[0][:, sl], op=op0)
                    nc.vector.tensor_tensor(bim[:, sl], x[1][:, sl], y[1][:, sl], op=op1)

            def emit_kblock(s, kb):
                c0 = 128 * kb
                kwid = 128 if kb < 4 else 8
                pslc = slice(0, kwid)
                xt = {}
                for h in range(2):
                    xt[("re", h)] = psp.tile(
                        [128, 512], f32, tag=f"xre{h}", name=f"xre{h}_{kb}"
                    )
                    xt[("im", h)] = psp.tile(
                        [128, 512], f32, tag=f"xim{h}", name=f"xim{h}_{kb}"
                    )
                for j in range(4):
                    h, o = j // 2, 256 * (j % 2)
                    xre = xt[("re", h)][pslc, o:o + 256]
                    xim = xt[("im", h)][pslc, o:o + 256]
                    for c in range(2):
                        bre, bim = s["B"][(j, c)]
                        first, last = c == 0, c == 1
                        psr = bre[:, c0:c0 + kwid]
                        psi = bim[:, c0:c0 + kwid]
                        nc.tensor.matmul(xre, psr, TT[(j, "c", c)][:], start=first, stop=False)
                        nc.tensor.matmul(xim, psi, TT[(j, "c", c)][:], start=first, stop=False)
                        nc.tensor.matmul(xre, psi, TT[(j, "s", c)][:], start=False, stop=last)
                        nc.tensor.matmul(xim, psr, TT[(j, "sn", c)][:], start=False, stop=last)
                chi_t = chip.tile([128, N], f32, tag="chi")
                for h in range(2):
                    sqr = sqp.tile([128, 512], f32, tag=f"sqr{h}")
                    sqi = sqp.tile([128, 512], f32, tag=f"sqi{h}")
                    nc.scalar.square(sqr[pslc, :], xt[("re", h)][pslc, :])
                    nc.scalar.square(sqi[pslc, :], xt[("im", h)][pslc, :])
                    for jh in range(2):
                        j = 2 * h + jh
                        o = 256 * jh
                        cap = chi_t[pslc, :]
                        strided = bass.AP(
                            cap.tensor, cap.offset + j, [cap.ap[0], [4, 256]]
                        )
                        nc.gpsimd.tensor_add(
                            strided, sqr[pslc, o:o + 256], sqi[pslc, o:o + 256]
                        )
                s["chi"][kb] = chi_t

            def emit_store_direct(s, kb):
                b = s["b"]
                chi_t = s["chi"][kb]
                if kb < 4:
                    nc.sync.dma_start(out[b, 512 + 128 * kb:640 + 128 * kb, :], chi_t[:])
                else:
                    nc.sync.dma_start(out[b, 0:1, :], chi_t[0:1, :])

            def emit_mirror(s, kb):
                # chi_rev = [chi[0], chi[1023..1]] then J-flip rows via PE
                chi_t = s["chi"][kb]
                cr = revp.tile([128, N], f32r, tag=f"rev{kb % 2}")
                cap = chi_t[:]
                nc.vector.tensor_copy(cr[:, 0:1], chi_t[:, 0:1])
                rsrc = bass.AP(cap.tensor, cap.offset + 1023, [cap.ap[0], [-1, 1023]])
                nc.vector.tensor_copy(cr[:, 1:1024], rsrc)
                mj = mjp.tile([128, N], f32, tag=f"mj{kb % 2}")
                for h in range(2):
                    jy = psp.tile([128, 512], f32, tag=f"xre{h}", name=f"jy{h}_{kb}")
                    nc.tensor.matmul(
                        jy[:], tJ[:], cr[:, 512 * h:512 * h + 512], start=True, stop=True
                    )
                    nc.scalar.copy(mj[:, 512 * h:512 * h + 512], jy[:])
                s["mj"] = s.get("mj", {})
                s["mj"][kb] = mj

            def emit_store_mirror(s, kb):
                # mj partition p holds k1 = 128*kb + 127 - p -> out row 385-128*kb+p
                b = s["b"]
                mj = s["mj"][kb]
                if kb == 0:
                    nc.sync.dma_start(out[b, 385:512, :], mj[0:127, :])
                else:
                    r0 = 385 - 128 * kb
                    nc.sync.dma_start(out[b, r0:r0 + 128, :], mj[:])

            # ---- schedule: sliced rbuild for batch 0 so PE starts early
            s0 = emit_load(0)
            for j in range(4):
                for kind in ("c", "s", "sn"):
                    load_tab(j, kind, 0, nc.sync)
                    load_tab(j, kind, 1, nc.sync)
            SA, SB = (0, 256), (256, KW)
            for c in range(4):
                emit_product(s0, c, *SA)
                emit_product(s0, c + 4, *SA)
                emit_L1(s0, c, *SA)
            emit_L2(s0, 0, *SA)
            emit_L2(s0, 1, *SA)
            emit_kblock(s0, 0)
            for c in range(4):
                emit_product(s0, c, *SB)
                emit_product(s0, c + 4, *SB)
                emit_L1(s0, c, *SB)
            emit_kblock(s0, 1)
            emit_mirror(s0, 0)
            emit_store_direct(s0, 0)
            emit_store_mirror(s0, 0)
            emit_L2(s0, 0, *SB)
            emit_L2(s0, 1, *SB)
            s1 = emit_load(1)
            emit_kblock(s0, 2)
            emit_mirror(s0, 1)
            emit_store_direct(s0, 1)
            emit_store_mirror(s0, 1)
            for c in (0, 4, 1, 5):
                emit_product(s1, c, 0, KW)
            emit_kblock(s0, 3)
            emit_mirror(s0, 2)
            emit_store_direct(s0, 2)
            emit_store_mirror(s0, 2)
            emit_L1(s1, 0, 0, KW)
            emit_L1(s1, 1, 0, KW)
            for c in (2, 6, 3, 7):
                emit_product(s1, c, 0, KW)
            emit_kblock(s0, 4)
            emit_mirror(s0, 3)
            emit_store_direct(s0, 3)
            emit_store_mirror(s0, 3)
            emit_store_direct(s0, 4)
            emit_L1(s1, 2, 0, KW)
            emit_L1(s1, 3, 0, KW)
            emit_L2(s1, 0, 0, KW)
            emit_L2(s1, 1, 0, KW)
            emit_kblock(s1, 0)
            emit_mirror(s1, 0)
            emit_store_direct(s1, 0)
            emit_store_mirror(s1, 0)
            emit_kblock(s1, 1)
            emit_mirror(s1, 1)
            emit_store_direct(s1, 1)
            emit_store_mirror(s1, 1)
            emit_kblock(s1, 2)
            emit_mirror(s1, 2)
            emit_store_direct(s1, 2)
            emit_store_mirror(s1, 2)
            emit_kblock(s1, 3)
            emit_mirror(s1, 3)
            emit_store_direct(s1, 3)
            emit_store_mirror(s1, 3)
            emit_kblock(s1, 4)
            emit_store_direct(s1, 4)

    _split_excess_waits(nc)
    return nc


_NC_CACHE = {}


def _get_nc():
    if "nc" not in _NC_CACHE:
        _NC_CACHE["nc"] = build_nc()
    return _NC_CACHE["nc"]


def _get_tables():
    if "tabs" not in _NC_CACHE:
        m = np.arange(256, dtype=np.float64)[:, None]
        tc_ = np.arange(256, dtype=np.float64)[None, :]
        t_of = (tc_ + 128) % 256
        tabs = {}
        for j in range(4):
            ang = 2.0 * np.pi * (m * j / 1024.0 + (m * t_of) % 256 / 256.0)
            tabs[f"t{j}c"] = np.cos(ang).astype(bf16np)
            tabs[f"t{j}s"] = np.sin(ang).astype(bf16np)
            tabs[f"t{j}sn"] = (-np.sin(ang)).astype(bf16np)
        _NC_CACHE["tabs"] = (tabs, np.eye(128, dtype=np.float32)[::-1].copy())
    return _NC_CACHE["tabs"]


def _make_in_maps(s_real, s_imag):
    s_real = np.asarray(s_real, dtype=np.float32)
    s_imag = np.asarray(s_imag, dtype=np.float32)
    tabs, jnp_ = _get_tables()
    E = (
        s_real.astype(np.float64) ** 2 + s_imag.astype(np.float64) ** 2
    ).sum(axis=1, keepdims=True)
    scale = E ** -0.5
    srn = (s_real * scale).astype(np.float32)
    sin_ = (s_imag * scale).astype(np.float32)

    in_maps = []
    for core in range(NCORES):
        sl = slice(core * BPC, (core + 1) * BPC)
        sr = srn[sl]
        si = sin_[sl]
        dsr = np.tile(sr, (1, 2)).astype(bf16np)
        dD = np.tile(si - sr, (1, 2)).astype(bf16np)
        dSn = np.tile(-(sr + si), (1, 2)).astype(bf16np)
        scols = np.concatenate(
            [
                sr.reshape(BPC, 8, 128).transpose(0, 2, 1),
                si.reshape(BPC, 8, 128).transpose(0, 2, 1),
                (sr + si).reshape(BPC, 8, 128).transpose(0, 2, 1),
            ],
            axis=2,
        ).astype(np.float32).copy()
        im = {"dsr": dsr, "dD": dD, "dSn": dSn, "scols": scols, "jmat": jnp_}
        im.update(tabs)
        in_maps.append(im)
    return in_maps


def kernel(s_real: np.ndarray, s_imag: np.ndarray) -> np.ndarray:
    nc = _get_nc()
    in_maps = _make_in_maps(s_real, s_imag)
    res = bass_utils.run_bass_kernel_spmd(nc, in_maps, core_ids=list(range(NCORES)))
    return np.concatenate([r["out"] for r in res.results], axis=0)


# revision 18
# speedup vs baseline: 1.2071x; 1.0753x over previous
"""Radix-4 DIF ambiguity kernel, bf16 lag products, host-folded normalization.

X[k, 4t+j] = sum_{m<256} B_j[m, k] * (w1024^{jm} w256^{mt})   (tables, bf16)
B_0 =  (R0+R2) + (R1+R3)     B_2 = (R0+R2) - (R1+R3)
B_1 =  (R0-R2) - i(R1-R3)    B_3 = (R0-R2) + i(R1-R3)     (Rl = R[m+256l])

The ambiguity max is always at the origin (Cauchy-Schwarz), so the max-
normalization is folded into a host-side input scaling s -> s/sqrt(sum|s|^2).
Lag products via 3-op STT chains against host-precomputed difference/sum
buffers. Only k in [0,512] is computed; rows 1..511 come from the
chi(-k,-f) symmetry: f-reversal on DVE, k-reversal via J-flip matmul.
"""

import numpy as np
import ml_dtypes

import bass_rust
import concourse.bass as bass
import concourse.mybir as mybir
import concourse.tile as tile
import concourse.bass_utils as bass_utils

B, N = 16, 1024
NCORES = 8
BPC = B // NCORES
KW = 520  # k-width computed: kb0-3 full, kb4 holds k=512 (+7 pad)

f32 = mybir.dt.float32
f32r = mybir.dt.float32r
bf16 = mybir.dt.bfloat16
ALU = mybir.AluOpType

bf16np = ml_dtypes.bfloat16


def _split_excess_waits(nc):
    for f in nc.m.functions:
        for blk in f.blocks:
            insts = list(blk.instructions)
            new_insts = []
            changed = False
            for inst in insts:
                si = inst.sync_info
                waits = list(si.on_wait) if (si is not None and si.on_wait) else []
                keep_n = 0 if isinstance(inst, mybir.InstDrain) else 1
                if len(waits) > keep_n:
                    changed = True
                    extra = waits[: len(waits) - keep_n]
                    keep = waits[len(waits) - keep_n:]
                    for w in extra:
                        nop = mybir.InstNoOp(
                            name=nc.get_next_instruction_name(), ins=[], outs=[]
                        )
                        nop.engine = inst.engine
                        nop.sync_info = bass_rust.SyncInfo(on_wait=[w], on_update=[])
                        new_insts.append(nop)
                    inst.sync_info = bass_rust.SyncInfo(
                        on_wait=keep,
                        on_update=list(si.on_update) if si.on_update else [],
                    )
                new_insts.append(inst)
            if changed:
                blk.instructions = new_insts
    return nc


def build_nc():
    nc = bass.Bass("TRN2", target_bir_lowering=False, debug=False)

    dsr = nc.dram_tensor("dsr", [BPC, 2048], bf16, kind="ExternalInput")
    dD = nc.dram_tensor("dD", [BPC, 2048], bf16, kind="ExternalInput")
    dSn = nc.dram_tensor("dSn", [BPC, 2048], bf16, kind="ExternalInput")
    scols = nc.dram_tensor("scols", [BPC, 128, 24], f32, kind="ExternalInput")
    tabs = {}
    for j in range(4):
        for kind in ("c", "s", "sn"):
            nm = f"t{j}{kind}"
            tabs[(j, kind)] = nc.dram_tensor(nm, [256, 256], bf16, kind="ExternalInput")
    jmat = nc.dram_tensor("jmat", [128, 128], f32r, kind="ExternalInput")
    out = nc.dram_tensor("out", [BPC, N, N], f32, kind="ExternalOutput")

    with tile.TileContext(nc) as tc:
        with (
            tc.tile_pool(name="const", bufs=1) as constp,
            tc.tile_pool(name="win", bufs=2) as winp,
            tc.tile_pool(name="sm", bufs=2) as smp,
            tc.tile_pool(name="u", bufs=1) as up,
            tc.tile_pool(name="tmp", bufs=2) as tmpp,
            tc.tile_pool(name="r1", bufs=1) as r1p,
            tc.tile_pool(name="b2", bufs=2) as b2p,
            tc.tile_pool(name="sq", bufs=3) as sqp,
            tc.tile_pool(name="chi", bufs=6) as chip,
            tc.tile_pool(name="rev", bufs=2) as revp,
            tc.tile_pool(name="mj", bufs=2) as mjp,
            tc.tile_pool(name="ps", bufs=2, space="PSUM") as psp,
        ):
            tJ = constp.tile([128, 128], f32r, tag="jmat")
            nc.sync.dma_start(tJ[:], jmat[:])
            TT = {}
            for j in range(4):
                for kind in ("c", "s", "sn"):
                    for c in range(2):
                        TT[(j, kind, c)] = constp.tile(
                            [128, 256], bf16, tag=f"t{j}{kind}{c}",
                            name=f"tt{j}{kind}{c}",
                        )

            def load_tab(j, kind, c, eng):
                eng.dma_start(
                    TT[(j, kind, c)][:], tabs[(j, kind)][128 * c:128 * (c + 1), :]
                )

            def emit_load(b):
                s = {"b": b, "chi": {}, "u": {}, "Rs": {}, "Rd": {}, "B": {}}
                scol = smp.tile([128, 24], f32, tag="scol")
                nc.sync.dma_start(scol[:], scols[b])
                Tsr = winp.tile([128, 1921], bf16, tag="tsr")
                TD = winp.tile([128, 1921], bf16, tag="tD")
                TSn = winp.tile([128, 1921], bf16, tag="tSn")
                # row p = s_tiled[p : p+1921] (shift-by-1 rows via DRAM stride)
                nc.sync.dma_start(Tsr[:], bass.AP(dsr, b * 2048, [[1, 128], [1, 1921]]))
                nc.sync.dma_start(TD[:], bass.AP(dD, b * 2048, [[1, 128], [1, 1921]]))
                nc.sync.dma_start(TSn[:], bass.AP(dSn, b * 2048, [[1, 128], [1, 1921]]))
                s["scol"] = scol
                s["T"] = (Tsr, TD, TSn)
                return s

            def win(T, q8, lo, n):
                # [p, kk] -> s_tiled[1024 + 128*q8 + p - (lo+kk)], kk in [0, n)
                ap = T[:]
                return bass.AP(
                    ap.tensor, ap.offset + 1024 + 128 * q8 - lo, [ap.ap[0], [-1, n]]
                )

            def emit_product(s, q8, lo, hi):
                # u = s[m] * conj(s)[(m-k)%N]:
                #   t    = w_sr * (sr+si)_m          (ACT)
                #   u_re = w_(si-sr) * si_m + t      (DVE/Pool STT)
                #   u_im = w_-(sr+si) * sr_m + t
                Tsr, TD, TSn = s["T"]
                scol = s["scol"]
                n = hi - lo
                sr_c = scol[:, q8:q8 + 1]
                si_c = scol[:, 8 + q8:9 + q8]
                ss_c = scol[:, 16 + q8:17 + q8]
                t = tmpp.tile([128, KW], bf16, tag=f"t{q8 % 2}")
                nc.scalar.mul(t[:, lo:hi], win(Tsr, q8, lo, n), ss_c)
                if lo == 0:
                    ur = up.tile([128, KW], bf16, tag=f"ur{q8}")
                    ui = up.tile([128, KW], bf16, tag=f"ui{q8}")
                    s["u"][q8] = (ur, ui)
                ur, ui = s["u"][q8]
                nc.vector.scalar_tensor_tensor(
                    ur[:, lo:hi], win(TD, q8, lo, n), si_c, t[:, lo:hi],
                    op0=ALU.mult, op1=ALU.add,
                )
                nc.vector.scalar_tensor_tensor(
                    ui[:, lo:hi], win(TSn, q8, lo, n), sr_c, t[:, lo:hi],
                    op0=ALU.mult, op1=ALU.add,
                )

            def emit_L1(s, c, lo, hi):
                # Rsum/Rdiff over m vs m+512: pairs (c, c+4), c in 0..3  (Pool)
                u1r, u1i = s["u"][c]
                u2r, u2i = s["u"][c + 4]
                if lo == 0:
                    rsr = r1p.tile([128, KW], bf16, tag=f"rsr{c}")
                    rsi = r1p.tile([128, KW], bf16, tag=f"rsi{c}")
                    rdr = r1p.tile([128, KW], bf16, tag=f"rdr{c}")
                    rdi = r1p.tile([128, KW], bf16, tag=f"rdi{c}")
                    s["Rs"][c] = (rsr, rsi)
                    s["Rd"][c] = (rdr, rdi)
                (rsr, rsi), (rdr, rdi) = s["Rs"][c], s["Rd"][c]
                sl = slice(lo, hi)
                eng = nc.gpsimd if c < 2 else nc.vector
                eng.tensor_add(rsr[:, sl], u1r[:, sl], u2r[:, sl])
                eng.tensor_add(rsi[:, sl], u1i[:, sl], u2i[:, sl])
                eng.tensor_sub(rdr[:, sl], u1r[:, sl], u2r[:, sl])
                eng.tensor_sub(rdi[:, sl], u1i[:, sl], u2i[:, sl])

            def emit_L2(s, c, lo, hi):
                # B_j over m'' vs m''+256: pairs (c, c+2), c in 0..1  (DVE)
                asr, asi = s["Rs"][c]
                bsr, bsi = s["Rs"][c + 2]
                cdr, cdi = s["Rd"][c]
                ddr, ddi = s["Rd"][c + 2]
                sl = slice(lo, hi)
                for j, (x, y, op0, op1) in {
                    0: ((asr, asi), (bsr, bsi), ALU.add, ALU.add),
                    2: ((asr, asi), (bsr, bsi), ALU.subtract, ALU.subtract),
                    # B1 = c - i d: re = cr + di ; im = ci - dr
                    1: ((cdr, cdi), (ddi, ddr), ALU.add, ALU.subtract),
                    # B3 = c + i d: re = cr - di ; im = ci + dr
                    3: ((cdr, cdi), (ddi, ddr), ALU.subtract, ALU.add),
                }.items():
                    if lo == 0:
                        bre = b2p.tile([128, KW], bf16, tag=f"b{j}re{c}")
                        bim = b2p.tile([128, KW], bf16, tag=f"b{j}im{c}")
                        s["B"][(j, c)] = (bre, bim)
                    bre, bim = s["B"][(j, c)]
                    nc.vector.tensor_tensor(bre[:, sl], x[0][:, sl], y[0][:, sl], op=op0)
                    nc.vector.tensor_tensor(bim[:, sl], x[1][:, sl], y[1][:, sl], op=op1)

            def emit_kblock(s, kb):
                c0 = 128 * kb
                kwid = 128 if kb < 4 else 8
                pslc = slice(0, kwid)
                xt = {}
                for h in range(2):
                    xt[("re", h)] = psp.tile(
                        [128, 512], f32, tag=f"xre{h}", name=f"xre{h}_{kb}",
                        bufs=2 if h == 0 else 1,
                    )
                    xt[("im", h)] = psp.tile(
                        [128, 512], f32, tag=f"xim{h}", name=f"xim{h}_{kb}",
                        bufs=2 if h == 0 else 1,
                    )
                for j in range(4):
                    h, o = j // 2, 256 * (j % 2)
                    xre = xt[("re", h)][pslc, o:o + 256]
                    xim = xt[("im", h)][pslc, o:o + 256]
                    for c in range(2):
                        bre, bim = s["B"][(j, c)]
                        first, last = c == 0, c == 1
                        psr = bre[:, c0:c0 + kwid]
                        psi = bim[:, c0:c0 + kwid]
                        nc.tensor.matmul(xre, psr, TT[(j, "c", c)][:], start=first, stop=False)
                        nc.tensor.matmul(xim, psi, TT[(j, "c", c)][:], start=first, stop=False)
                        nc.tensor.matmul(xre, psi, TT[(j, "s", c)][:], start=False, stop=last)
                        nc.tensor.matmul(xim, psr, TT[(j, "sn", c)][:], start=False, stop=last)
                chi_t = chip.tile([128, N], f32, tag="chi")
                for h in range(2):
                    sqr = sqp.tile([128, 512], f32, tag=f"sqr{h}")
                    sqi = sqp.tile([128, 512], f32, tag=f"sqi{h}")
                    nc.scalar.square(sqr[pslc, :], xt[("re", h)][pslc, :])
                    nc.scalar.square(sqi[pslc, :], xt[("im", h)][pslc, :])
                    for jh in range(2):
                        j = 2 * h + jh
                        o = 256 * jh
                        cap = chi_t[pslc, :]
                        strided = bass.AP(
                            cap.tensor, cap.offset + j, [cap.ap[0], [4, 256]]
                        )
                        nc.gpsimd.tensor_add(
                            strided, sqr[pslc, o:o + 256], sqi[pslc, o:o + 256]
                        )
                s["chi"][kb] = chi_t

            def emit_store_direct(s, kb):
                b = s["b"]
                chi_t = s["chi"][kb]
                if kb < 4:
                    nc.sync.dma_start(out[b, 512 + 128 * kb:640 + 128 * kb, :], chi_t[:])
                else:
                    nc.sync.dma_start(out[b, 0:1, :], chi_t[0:1, :])

            def emit_mirror(s, kb):
                # chi_rev = [chi[0], chi[1023..1]] then J-flip rows via PE
                chi_t = s["chi"][kb]
                cr = revp.tile([128, N], f32r, tag=f"rev{kb % 2}")
                cap = chi_t[:]
                nc.vector.tensor_copy(cr[:, 0:1], chi_t[:, 0:1])
                rsrc = bass.AP(cap.tensor, cap.offset + 1023, [cap.ap[0], [-1, 1023]])
                nc.vector.tensor_copy(cr[:, 1:1024], rsrc)
                mj = mjp.tile([128, N], f32, tag=f"mj{kb % 2}")
                for h in range(2):
                    jy = psp.tile(
                        [128, 512], f32, tag="jy", name=f"jy{h}_{kb}", bufs=2
                    )
                    nc.tensor.matmul(
                        jy[:], tJ[:], cr[:, 512 * h:512 * h + 512], start=True, stop=True
                    )
                    nc.scalar.copy(mj[:, 512 * h:512 * h + 512], jy[:])
                s["mj"] = s.get("mj", {})
                s["mj"][kb] = mj

            def emit_store_mirror(s, kb):
                # mj partition p holds k1 = 128*kb + 127 - p -> out row 385-128*kb+p
                b = s["b"]
                mj = s["mj"][kb]
                if kb == 0:
                    nc.sync.dma_start(out[b, 385:512, :], mj[0:127, :])
                else:
                    r0 = 385 - 128 * kb
                    nc.sync.dma_start(out[b, r0:r0 + 128, :], mj[:])

            # ---- schedule: all loads up front, sliced rbuild for batch 0,
            # batch-1 rbuild overlapped with batch-0 kblocks, kb interleave
            s0 = emit_load(0)
            for j in range(4):
                for kind in ("c", "s", "sn"):
                    load_tab(j, kind, 0, nc.sync)
                    load_tab(j, kind, 1, nc.sync)
            s1 = emit_load(1)
            SA, SB = (0, 256), (256, KW)
            for c in range(4):
                emit_product(s0, c, *SA)
                emit_product(s0, c + 4, *SA)
                emit_L1(s0, c, *SA)
            emit_L2(s0, 0, *SA)
            emit_L2(s0, 1, *SA)
            emit_kblock(s0, 0)
            for c in range(4):
                emit_product(s0, c, *SB)
                emit_product(s0, c + 4, *SB)
                emit_L1(s0, c, *SB)
            emit_L2(s0, 0, *SB)
            emit_L2(s0, 1, *SB)
            emit_kblock(s0, 1)
            emit_mirror(s0, 0)
            emit_store_direct(s0, 0)
            emit_store_mirror(s0, 0)
            for c in (0, 4, 1, 5):
                emit_product(s1, c, 0, KW)
            emit_L1(s1, 0, 0, KW)
            emit_kblock(s0, 2)
            emit_mirror(s0, 1)
            emit_store_direct(s0, 1)
            emit_store_mirror(s0, 1)
            for c in (2, 6, 3, 7):
                emit_product(s1, c, 0, KW)
            emit_L1(s1, 1, 0, KW)
            emit_L1(s1, 2, 0, KW)
            emit_L1(s1, 3, 0, KW)
            emit_kblock(s0, 3)
            emit_mirror(s0, 2)
            emit_store_direct(s0, 2)
            emit_store_mirror(s0, 2)
            emit_L2(s1, 0, 0, KW)
            emit_L2(s1, 1, 0, KW)
            emit_kblock(s1, 0)
            emit_kblock(s0, 4)
            emit_mirror(s0, 3)
            emit_store_direct(s0, 3)
            emit_store_mirror(s0, 3)
            emit_store_direct(s0, 4)
            emit_kblock(s1, 1)
            emit_mirror(s1, 0)
            emit_store_direct(s1, 0)
            emit_store_mirror(s1, 0)
            emit_kblock(s1, 2)
            emit_mirror(s1, 1)
            emit_store_direct(s1, 1)
            emit_store_mirror(s1, 1)
            emit_kblock(s1, 3)
            emit_mirror(s1, 2)
            emit_store_direct(s1, 2)
            emit_store_mirror(s1, 2)
            emit_kblock(s1, 4)
            emit_mirror(s1, 3)
            emit_store_direct(s1, 3)
            emit_store_mirror(s1, 3)
            emit_store_direct(s1, 4)

    _split_excess_waits(nc)
    return nc


_NC_CACHE = {}


def _get_nc():
    if "nc" not in _NC_CACHE:
        _NC_CACHE["nc"] = build_nc()
    return _NC_CACHE["nc"]


def _get_tables():
    if "tabs" not in _NC_CACHE:
        m = np.arange(256, dtype=np.float64)[:, None]
        tc_ = np.arange(256, dtype=np.float64)[None, :]
        t_of = (tc_ + 128) % 256
        tabs = {}
        for j in range(4):
            ang = 2.0 * np.pi * (m * j / 1024.0 + (m * t_of) % 256 / 256.0)
            tabs[f"t{j}c"] = np.cos(ang).astype(bf16np)
            tabs[f"t{j}s"] = np.sin(ang).astype(bf16np)
            tabs[f"t{j}sn"] = (-np.sin(ang)).astype(bf16np)
        _NC_CACHE["tabs"] = (tabs, np.eye(128, dtype=np.float32)[::-1].copy())
    return _NC_CACHE["tabs"]


def _make_in_maps(s_real, s_imag):
    s_real = np.asarray(s_real, dtype=np.float32)
    s_imag = np.asarray(s_imag, dtype=np.float32)
    tabs, jnp_ = _get_tables()
    E = (
        s_real.astype(np.float64) ** 2 + s_imag.astype(np.float64) ** 2
    ).sum(axis=1, keepdims=True)
    scale = E ** -0.5
    srn = (s_real * scale).astype(np.float32)
    sin_ = (s_imag * scale).astype(np.float32)

    in_maps = []
    for core in range(NCORES):
        sl = slice(core * BPC, (core + 1) * BPC)
        sr = srn[sl]
        si = sin_[sl]
        dsr = np.tile(sr, (1, 2)).astype(bf16np)
        dD = np.tile(si - sr, (1, 2)).astype(bf16np)
        dSn = np.tile(-(sr + si), (1, 2)).astype(bf16np)
        scols = np.concatenate(
            [
                sr.reshape(BPC, 8, 128).transpose(0, 2, 1),
                si.reshape(BPC, 8, 128).transpose(0, 2, 1),
                (sr + si).reshape(BPC, 8, 128).transpose(0, 2, 1),
            ],
            axis=2,
        ).astype(np.float32).copy()
        im = {"dsr": dsr, "dD": dD, "dSn": dSn, "scols": scols, "jmat": jnp_}
        im.update(tabs)
        in_maps.append(im)
    return in_maps


def kernel(s_real: np.ndarray, s_imag: np.ndarray) -> np.ndarray:
    nc = _get_nc()
    in_maps = _make_in_maps(s_real, s_imag)
    res = bass_utils.run_bass_kernel_spmd(nc, in_maps, core_ids=list(range(NCORES)))
    return np.concatenate([r["out"] for r in res.results], axis=0)
